# revision 1
# baseline (speedup 1.0000x reference)
"""Trainium2 Bass kernel for nn_Classifier_86260123173820 (GNN message passing).

Strategy (8 NeuronCores, pure data parallelism, 8 graphs per core):
  - Per graph, the 4 message-passing layers apply a fixed sparse operator
    (segment_sum over edges + residual).  We fold the residual into a dense
    augmented adjacency  A_aug = A + I  (integer counts <= 4, exact in fp8)
    and evaluate  pooled = A_aug @ q  on the PE as a dense matmul with the
    2048x2048 A_aug^T streamed as the moving operand (fp8) against a
    stationary operand holding a bf16 TRIPLE-split of q (hi/lo/lo2), giving
    fp32-accurate products with fp32 PSUM accumulation.  This keeps the
    sort-pooling channel accurate to ~1e-7, which measurement shows is
    required for the top-k selection to match the fp32 reference.
  - e2n (edge_feat segment_sum) is done from a host-padded, row-sorted
    edge tensor with a strided DVE reduce (fp32, edge order preserved).
  - Sort-pooling top-30 per graph uses the DVE max8/max_index/match_replace
    iteration (exactly matches argsort on distinct values).
  - The conv/MLP head runs batched across the core's 8 graphs on PE/DVE/ACT.
"""

import numpy as np
import ml_dtypes

B, NPG, DEG = 64, 2048, 16
N, E = B * NPG, B * NPG * DEG
NF, EF = 128, 32
K = 30
TLD = 97
C1, C2, KW2 = 16, 32, 5
HID, NCLS = 128, 10
NCORES = 8
GPC = B // NCORES          # graphs per core
EPG = NPG * DEG            # edges per graph
NCH = NPG // 128           # 128-node chunks per graph (16)

_cache = {}


def _build_program(S):
    import concourse.bass as bass
    import concourse.bacc as bacc
    import concourse.mybir as mybir
    import concourse.tile as tile
    from concourse.masks import make_identity
    dt = mybir.dt
    AF = mybir.ActivationFunctionType
    OP = mybir.AluOpType
    AX = mybir.AxisListType

    nc = bacc.Bacc("TRN2", target_bir_lowering=False, debug=False,
                   num_devices=NCORES)

    # ---- inputs (per core) ----
    d_nfT = nc.dram_tensor("nfT", [GPC, NF, NPG], dt.float32, kind="ExternalInput")
    d_efp = nc.dram_tensor("efp", [GPC, NCH, 128, EF, S], dt.float32, kind="ExternalInput")
    d_at8 = nc.dram_tensor("at8", [GPC, 128, NCH, NPG], dt.float8e4, kind="ExternalInput")
    d_degb = nc.dram_tensor("degb", [GPC, 32, NPG], dt.float32, kind="ExternalInput")
    d_w0a = nc.dram_tensor("w0aT", [NF, 32], dt.float32, kind="ExternalInput")
    d_w0b = nc.dram_tensor("w0bT", [EF, 32], dt.float32, kind="ExternalInput")
    d_w1 = nc.dram_tensor("w1T", [32, 32], dt.float32, kind="ExternalInput")
    d_w2 = nc.dram_tensor("w2T", [32, 32], dt.float32, kind="ExternalInput")
    d_w3 = nc.dram_tensor("w3T", [32, 1], dt.float32, kind="ExternalInput")
    d_b0 = nc.dram_tensor("b0c", [32, 1], dt.float32, kind="ExternalInput")
    d_b1 = nc.dram_tensor("b1c", [32, 1], dt.float32, kind="ExternalInput")
    d_b2 = nc.dram_tensor("b2c", [32, 1], dt.float32, kind="ExternalInput")
    d_b3 = nc.dram_tensor("b3c", [1, 1], dt.float32, kind="ExternalInput")
    d_wc1 = nc.dram_tensor("wc1T", [TLD, C1], dt.float32, kind="ExternalInput")
    d_wc2 = nc.dram_tensor("wc2T", [C1, KW2, C2], dt.float32, kind="ExternalInput")
    d_bc1 = nc.dram_tensor("bc1c", [C1, 1], dt.float32, kind="ExternalInput")
    d_bc2 = nc.dram_tensor("bc2c", [C2, 1], dt.float32, kind="ExternalInput")
    d_wh = nc.dram_tensor("whT", [C2, 11, HID], dt.float32, kind="ExternalInput")
    d_bh = nc.dram_tensor("bhc", [HID, 1], dt.float32, kind="ExternalInput")
    d_wo = nc.dram_tensor("woT", [HID, NCLS], dt.float32, kind="ExternalInput")
    d_bo = nc.dram_tensor("boc", [NCLS, 1], dt.float32, kind="ExternalInput")

    d_cat = nc.dram_tensor("catd", [GPC * NPG, TLD], dt.float32)  # internal
    d_tis = nc.dram_tensor("tis", [GPC, 32], dt.uint32)  # internal scratch
    d_out = nc.dram_tensor("out", [GPC, NCLS], dt.float32, kind="ExternalOutput")

    LAYERS = [(d_w1, d_b1, 32), (d_w2, d_b2, 32), (d_w3, d_b3, 1)]

    with tile.TileContext(nc) as tc:
        with (
            tc.tile_pool(name="pw", bufs=1) as pw,         # persistent weights
            tc.tile_pool(name="pA", bufs=2) as pA,          # A^T fp8, per graph
            tc.tile_pool(name="pNF", bufs=2) as pNF,        # nfT per graph
            tc.tile_pool(name="pEF", bufs=3) as pEF,        # efp chunks
            tc.tile_pool(name="pG", bufs=2) as pG,          # cat (dma-out overlap)
            tc.tile_pool(name="pE2", bufs=1) as pE2,        # e2nT
            tc.tile_pool(name="pL", bufs=1) as pL,          # per-layer transients
            tc.tile_pool(name="pS", bufs=1) as pS,          # sortbuf & head
            tc.tile_pool(name="psQ", bufs=2, space="PSUM") as psQ,
            tc.tile_pool(name="psP", bufs=1, space="PSUM") as psP,
            tc.tile_pool(name="psT", bufs=2, space="PSUM") as psT,
        ):
            # persistent small tensors
            ident = pw.tile([128, 128], dt.float32)
            make_identity(nc, ident[:])
            w0a = pw.tile([NF, 32], dt.float32)
            w0b = pw.tile([EF, 32], dt.float32)
            nc.sync.dma_start(w0a[:], d_w0a.ap())
            nc.sync.dma_start(w0b[:], d_w0b.ap())
            wl = []
            for dW, dB, w in LAYERS:
                tW = pw.tile([32, w], dt.float32, tag=f"w{w}{len(wl)}")
                tB = pw.tile([w, 1], dt.float32, tag=f"b{w}{len(wl)}")
                nc.sync.dma_start(tW[:], dW.ap())
                nc.sync.dma_start(tB[:], dB.ap())
                wl.append((tW, tB, w))
            b0 = pw.tile([32, 1], dt.float32)
            nc.sync.dma_start(b0[:], d_b0.ap())
            wc1 = pw.tile([TLD, C1], dt.float32)
            wc2 = pw.tile([C1, KW2, C2], dt.float32)
            bc1 = pw.tile([C1, 1], dt.float32)
            bc2 = pw.tile([C2, 1], dt.float32)
            wh = pw.tile([C2, 11, HID], dt.float32)
            bh = pw.tile([HID, 1], dt.float32)
            wo = pw.tile([HID, NCLS], dt.float32)
            bo = pw.tile([NCLS, 1], dt.float32)
            for t, d in ((wc1, d_wc1), (wc2, d_wc2), (bc1, d_bc1), (bc2, d_bc2),
                         (wh, d_wh), (bh, d_bh), (wo, d_wo), (bo, d_bo)):
                nc.sync.dma_start(t[:], d.ap())

            sortbuf = pS.tile([GPC, NPG], dt.float32)
            cat_dmas = []

            for g in range(GPC):
                at = pA.tile([128, NCH, NPG], dt.float8e4, tag="at")
                nc.sync.dma_start(at[:], d_at8.ap()[g])
                nfT = pNF.tile([NF, NPG], dt.float32, tag="nf")
                nc.sync.dma_start(nfT[:], d_nfT.ap()[g])
                degb = pNF.tile([32, NPG], dt.float32, tag="deg")
                nc.sync.dma_start(degb[:], d_degb.ap()[g])

                # ---- e2n: strided reduce of padded sorted edge features ----
                e2nT = pE2.tile([EF, NPG], dt.float32, tag="e2nT")
                for ch in range(NCH):
                    ef_t = pEF.tile([128, EF, S], dt.float32, tag="ef")
                    nc.sync.dma_start(ef_t[:], d_efp.ap()[g, ch])
                    red = pL.tile([128, EF], dt.float32, tag="red")
                    nc.vector.tensor_reduce(red[:], ef_t[:], axis=AX.X, op=OP.add)
                    ptr = psT.tile([EF, 128], dt.float32, tag="tr")
                    nc.tensor.transpose(ptr[:], red[:], ident[:])
                    nc.vector.tensor_copy(e2nT[:, ch * 128:(ch + 1) * 128], ptr[:])

                # ---- q1 = [nf, e2n] @ W0^T  (node-major, fp32) ----
                qnm = psQ.tile([128, NCH * 32], dt.float32, tag="qnm")
                for ch in range(NCH):
                    sl = slice(ch * 128, (ch + 1) * 128)
                    o = qnm[:, ch * 32:(ch + 1) * 32]
                    nc.tensor.matmul(o, nfT[:, sl], w0a[:], start=True, stop=False)
                    nc.tensor.matmul(o, e2nT[:, sl], w0b[:], start=False, stop=True)

                catn = pG.tile([128, NCH, TLD], dt.float32, tag="cat")
                hT = None
                bias = b0
                for li in range(4):
                    w = 32 if li < 3 else 1
                    # split q (psum fp32 node-major [128, NCH*w]) into bf16 x3
                    # splits live at stationary cols 0/32/64 (PSUM rows stay
                    # 32-aligned even for the width-1 layer)
                    qs = pL.tile([128, NCH, 96], dt.bfloat16, tag="qs")
                    if w == 1:
                        nc.gpsimd.memset(qs[:], 0.0)
                    qf = pL.tile([128, NCH * w], dt.float32, tag="qf")
                    nc.vector.tensor_copy(qf[:], qnm[:, :NCH * w])
                    t1 = pL.tile([128, NCH * w], dt.float32, tag="t1")
                    t2 = pL.tile([128, NCH * w], dt.float32, tag="t2")
                    qv = qf[:].rearrange("p (c j) -> p c j", j=w)
                    t1v = t1[:].rearrange("p (c j) -> p c j", j=w)
                    t2v = t2[:].rearrange("p (c j) -> p c j", j=w)
                    nc.vector.tensor_copy(qs[:, :, 0:w], qv)
                    nc.vector.tensor_copy(t1v, qs[:, :, 0:w])
                    nc.vector.tensor_tensor(t2[:], qf[:], t1[:], op=OP.subtract)
                    nc.vector.tensor_copy(qs[:, :, 32:32 + w], t2v)
                    nc.vector.tensor_copy(t1v, qs[:, :, 32:32 + w])
                    nc.vector.tensor_tensor(t2[:], t2[:], t1[:], op=OP.subtract)
                    nc.vector.tensor_copy(qs[:, :, 64:64 + w], t2v)

                    # ---- big stream: pooledT = A_aug^T-stream vs q-splits ----
                    pT = psP.tile([96, NPG], dt.float32, tag="pT")
                    for nch in range(4):
                        o = pT[:, nch * 512:(nch + 1) * 512]
                        for kt in range(NCH):
                            nc.tensor.matmul(
                                o, qs[:, kt, :], at[:, kt, nch * 512:(nch + 1) * 512],
                                start=(kt == 0), stop=(kt == NCH - 1))

                    # ---- epilogue (f-major): sum splits, +b, /deg, tanh ----
                    s1 = pL.tile([w, NPG], dt.float32, tag="s1")
                    sA = pL.tile([w, NPG], dt.float32, tag="sA")
                    sB = pL.tile([w, NPG], dt.float32, tag="sB")
                    nc.vector.tensor_copy(s1[:], pT[0:w, :])
                    nc.vector.tensor_copy(sA[:], pT[32:32 + w, :])
                    nc.vector.tensor_copy(sB[:], pT[64:64 + w, :])
                    nc.vector.tensor_tensor(s1[:], s1[:], sA[:], op=OP.add)
                    nc.vector.tensor_tensor(s1[:], s1[:], sB[:], op=OP.add)
                    nc.vector.tensor_scalar(s1[:], s1[:], bias[:], None, op0=OP.add)
                    nc.vector.tensor_tensor(s1[:], s1[:], degb[0:w, :], op=OP.mult)
                    hT = pL.tile([w, NPG], dt.float32, tag="hT")
                    nc.scalar.activation(hT[:], s1[:], AF.Tanh)

                    # ---- cat columns (node-major) via PE transpose ----
                    off = 32 * li
                    for ch in range(NCH):
                        ptr = psT.tile([128, w], dt.float32, tag="tr")
                        nc.tensor.transpose(ptr[:], hT[:, ch * 128:(ch + 1) * 128], ident[0:w, 0:w])
                        nc.vector.tensor_copy(catn[:, ch, off:off + w], ptr[:])

                    if li < 3:
                        tW, tB, wn = wl[li]
                        bias = tB
                        qnm = psQ.tile([128, NCH * wn], dt.float32, tag="qnm")
                        for ch in range(NCH):
                            nc.tensor.matmul(qnm[:, ch * wn:(ch + 1) * wn],
                                             hT[:, ch * 128:(ch + 1) * 128],
                                             tW[:], start=True, stop=True)

                # sort channel row for this graph: DMA f-major h4 into sortbuf[g]
                nc.sync.dma_start(sortbuf[g:g + 1, :], hT[:])

                # cat -> DRAM (node-major [2048, 97])
                cat_dmas.append(nc.sync.dma_start(
                    d_cat.ap().rearrange("(g c p) d -> g p c d", g=GPC, p=128)[g],
                    catn[:]).ins)

            # ---- sortpooling: top-30 (+2 spare) per graph ----
            tv = pS.tile([GPC, 32], dt.float32)
            ti = pS.tile([GPC, 32], dt.uint32)
            for r in range(4):
                nc.vector.max(tv[:, r * 8:(r + 1) * 8], sortbuf[:])
                nc.vector.max_index(ti[:, r * 8:(r + 1) * 8], tv[:, r * 8:(r + 1) * 8], sortbuf[:])
                if r < 3:
                    nc.vector.match_replace(sortbuf[:], tv[:, r * 8:(r + 1) * 8], sortbuf[:], -1e30)

            # transpose indices to column-major [32, GPC] for indirect DMA
            # (tiny SBUF->SBUF DMAs; avoids int<->float conversion subtleties)
            tiTu = pS.tile([32, GPC], dt.uint32)
            nc.sync.dma_start(d_tis.ap(), ti[:])
            nc.sync.dma_start(tiTu[:], d_tis.ap().rearrange("a b -> b a"))

            # ---- head, batched over the core's graphs ----
            rhs_all = pS.tile([TLD, GPC * K], dt.float32)
            for g in range(GPC):
                pg = pS.tile([K, TLD], dt.float32, tag="pg")
                gi = nc.gpsimd.indirect_dma_start(
                    out=pg[:], out_offset=None, in_=d_cat.ap(),
                    in_offset=__import__("concourse.bass", fromlist=["IndirectOffsetOnAxis"]).IndirectOffsetOnAxis(ap=tiTu[0:K, g:g + 1], axis=0),
                    element_offset=g * NPG * TLD)
                from concourse.tile_rust import add_dep_helper as _adh
                for _cd in cat_dmas:
                    _adh(gi.ins, _cd, reason="gather after cat writes")
                ppg = psT.tile([TLD, K], dt.float32, tag="tr")
                nc.tensor.transpose(ppg[:], pg[:], ident[0:K, 0:K])
                nc.vector.tensor_copy(rhs_all[:, g * K:(g + 1) * K], ppg[:])

            ps1 = psQ.tile([C1, GPC * K], dt.float32, tag="qnm")
            nc.tensor.matmul(ps1[:], wc1[:], rhs_all[:], start=True, stop=True)
            y1 = pS.tile([C1, GPC * K], dt.float32)
            nc.scalar.activation(y1[:], ps1[:], AF.Relu, bias=bc1[:])
            y1v = y1[:].rearrange("a (g k t) -> a g k t", g=GPC, t=2)
            y2 = pS.tile([C1, GPC, K // 2], dt.float32)
            nc.vector.tensor_tensor(y2[:], y1v[:, :, :, 0], y1v[:, :, :, 1], op=OP.max)

            ps2 = psQ.tile([C2, GPC * 11], dt.float32, tag="qnm")
            r2t = pS.tile([C1, GPC * 11], dt.float32, tag="r2t")
            for t in range(KW2):
                r2tv = r2t[:].rearrange("a (g j) -> a g j", g=GPC)
                nc.vector.tensor_copy(r2tv, y2[:, :, t:t + 11])
                nc.tensor.matmul(ps2[:], wc2[:, t, :], r2t[:], start=(t == 0), stop=(t == KW2 - 1))
            y3 = pS.tile([C2, GPC * 11], dt.float32)
            nc.scalar.activation(y3[:], ps2[:], AF.Relu, bias=bc2[:])
            y3v = y3[:].rearrange("a (g j) -> a g j", g=GPC)

            psh = psQ.tile([HID, GPC], dt.float32, tag="qnm")
            r3t = pS.tile([C2, GPC], dt.float32, tag="r3t")
            for j in range(11):
                nc.vector.tensor_copy(r3t[:], y3v[:, :, j])
                nc.tensor.matmul(psh[:], wh[:, j, :], r3t[:], start=(j == 0), stop=(j == 10))
            h1 = pS.tile([HID, GPC], dt.float32)
            nc.scalar.activation(h1[:], psh[:], AF.Relu, bias=bh[:])

            psl = psQ.tile([NCLS, GPC], dt.float32, tag="qnm")
            nc.tensor.matmul(psl[:], wo[:], h1[:], start=True, stop=True)
            lg0 = pS.tile([NCLS, GPC], dt.float32)
            nc.scalar.activation(lg0[:], psl[:], AF.Identity, bias=bo[:])
            plT = psT.tile([GPC, NCLS], dt.float32, tag="tr")
            nc.tensor.transpose(plT[:], lg0[:], ident[0:NCLS, 0:NCLS])
            lgT = pS.tile([GPC, NCLS], dt.float32)
            nc.vector.tensor_copy(lgT[:], plT[:])

            mx = pS.tile([GPC, 1], dt.float32)
            nc.vector.tensor_reduce(mx[:], lgT[:], axis=AX.X, op=OP.max)
            sh = pS.tile([GPC, NCLS], dt.float32)
            nc.vector.tensor_scalar(sh[:], lgT[:], mx[:], None, op0=OP.subtract)
            ex = pS.tile([GPC, NCLS], dt.float32)
            sm = pS.tile([GPC, 1], dt.float32)
            nc.scalar.activation(ex[:], sh[:], AF.Exp, accum_out=sm[:])
            lsm = pS.tile([GPC, 1], dt.float32)
            nc.scalar.activation(lsm[:], sm[:], AF.Ln)
            osm = pS.tile([GPC, NCLS], dt.float32)
            nc.vector.tensor_scalar(osm[:], sh[:], lsm[:], None, op0=OP.subtract)
            nc.sync.dma_start(d_out.ap(), osm[:])

    nc.compile()
    return nc


def _prep_inputs(node_feat, edge_feat, node_degs, W0, b0, W1, b1, W2, b2, W3, b3,
                 Wc1, bc1, Wc2, bc2, Wh, bh, Wo, bo, edge_row, edge_col):
    """Host-side index preprocessing + per-core input maps."""
    bf8 = ml_dtypes.float8_e4m3

    deg_all = np.bincount(edge_row, minlength=N)
    maxdeg = int(deg_all.max())
    S = max(32, ((maxdeg + 7) // 8) * 8)

    # shared weight tensors
    shared = {
        "w0aT": np.ascontiguousarray(W0[:, :NF].T),
        "w0bT": np.ascontiguousarray(W0[:, NF:].T),
        "w1T": np.ascontiguousarray(W1.T), "w2T": np.ascontiguousarray(W2.T),
        "w3T": np.ascontiguousarray(W3.T),
        "b0c": b0.reshape(32, 1), "b1c": b1.reshape(32, 1),
        "b2c": b2.reshape(32, 1), "b3c": b3.reshape(1, 1),
        "wc1T": np.ascontiguousarray(Wc1.T),
        "wc2T": np.ascontiguousarray(Wc2.transpose(1, 2, 0)),  # [C1, KW2, C2]
        "bc1c": bc1.reshape(C1, 1), "bc2c": bc2.reshape(C2, 1),
        "whT": np.ascontiguousarray(Wh.reshape(HID, C2, 11).transpose(1, 2, 0)),  # [C2, 11, HID]
        "bhc": bh.reshape(HID, 1),
        "woT": np.ascontiguousarray(Wo.T), "boc": bo.reshape(NCLS, 1),
    }
    shared = {k: v.astype(np.float32) for k, v in shared.items()}

    in_maps = []
    for c in range(NCORES):
        gs = range(c * GPC, (c + 1) * GPC)
        nfT = np.empty((GPC, NF, NPG), np.float32)
        efp = np.zeros((GPC, NCH, 128, EF, S), np.float32)
        at8 = np.empty((GPC, 128, NCH, NPG), bf8)
        degb = np.empty((GPC, 32, NPG), np.float32)
        for i, g in enumerate(gs):
            nsl = slice(g * NPG, (g + 1) * NPG)
            esl = slice(g * EPG, (g + 1) * EPG)
            er = edge_row[esl] - g * NPG
            ec = edge_col[esl] - g * NPG
            nfT[i] = node_feat[nsl].T
            degb[i] = np.broadcast_to((np.float32(1.0) / node_degs[nsl].astype(np.float32)).reshape(1, NPG), (32, NPG))
            # A_aug^T: [m, n] = count(col=m, row=n) + I
            cnt = np.bincount(ec.astype(np.int64) * NPG + er, minlength=NPG * NPG)
            A = cnt.reshape(NPG, NPG).astype(np.float32)
            A[np.arange(NPG), np.arange(NPG)] += 1.0
            assert A.max() <= 16, "fp8 e4m3 exact-count range exceeded"
            at8[i] = A.reshape(NCH, 128, NPG).transpose(1, 0, 2).astype(bf8)
            # padded row-sorted edge features: [node, feat, slot]
            order = np.argsort(er, kind="stable")
            sr = er[order]
            deg = np.bincount(sr, minlength=NPG)
            starts = np.zeros(NPG, np.int64)
            starts[1:] = np.cumsum(deg)[:-1]
            pos = np.arange(EPG) - starts[sr]
            ef_g = edge_feat[esl][order]
            dst = efp[i].reshape(NPG, EF, S)
            dst[sr, :, pos] = ef_g
        m = dict(shared)
        m.update(nfT=nfT, efp=efp, at8=at8, degb=degb)
        in_maps.append(m)
    return in_maps, S


def kernel(**inputs):
    from concourse.bass_utils import run_bass_kernel_spmd
    in_maps, S = _prep_inputs(**inputs)
    if ("nc", S) not in _cache:
        _cache[("nc", S)] = _build_program(S)
    nc = _cache[("nc", S)]
    res = run_bass_kernel_spmd(nc, in_maps, core_ids=list(range(NCORES)))
    out = np.concatenate([res.results[c]["out"] for c in range(NCORES)], axis=0)
    return out.astype(np.float32)

